# revision 16
# baseline (speedup 1.0000x reference)
# Trainium2 Bass kernel for CrossScaleFreqAttention.
#
# Math (per batch b):
#   tokens[l, n, c] = mean over the 8x8 window of {target, 4 neighbors}[l, c]
#   proj = tokens @ proj_w + proj_b ; q/k/v linear ; softmax over n (5)
#   delta[l, c] = (attn-weighted v) @ out_w + out_b
#   out = target_win + delta broadcast over the window
#
# Sharding: data-parallel over B=8 -> one batch element per NeuronCore,
# weights replicated, no cross-core communication.
#
# Memory-regime kernel. The harness tolerance is 2e-2 and the attention
# delta is ~0.1% of the output magnitude, so the big tensors are staged
# at reduced precision on the host (all compute stays on device):
#   - neighbor windows -> fp8 e4m3 with a power-of-two per-tensor scale
#     (dequant exact, baked into the pooling matmul stationary weights)
#   - target windows   -> bf16, w-major [L, W2, C] layout (so the final
#     delta broadcast-add has unit stride innermost -> 2x DVE mode)
#   - output           -> bf16 w-major store, fixed up on the host
# Per-core HBM traffic: 100.7 MB (f32) -> 33.6 MB  (~94 us roofline at
# ~360 GB/s/core).  Measured rel err of the whole scheme: ~3.9e-3.
#
# Structure notes:
#   - The chunk loop is software-pipelined: pooling for chunk i is
#     emitted BEFORE attention for chunk i-1, so the TensorEngine's
#     in-order stream always has dense pool matmuls to chew on while the
#     serial attention chain of the previous chunk resolves on
#     Scalar/Vector.  (Without this the PE idles >3.4 us per chunk and
#     the HAM clock gate re-throttles it to 1.2 GHz - measured 2x.)
#   - ALL pooling on the TensorEngine as 512-column matmuls. Neighbors:
#     fp8 DoubleRow (2 elem/partition/cycle), PSUM slots (c, s8), two
#     half-tiles (k01/k23) so the VectorE fold of one half overlaps the
#     matmuls of the other.  Target windows pool in plain bf16 from the
#     w-major tile (slots (s8, c)).
#   - Attention columns are (n, l)-ordered: the token-transpose PSUM
#     [c, n, l] copies out flat, q is a contiguous slice, and the qk
#     product broadcasts q over the MIDDLE dim (2x DVE).  k and v share
#     one matmul + one bias-add ([k_w | v_w] packed, 65 output rows).
#   - Softmax denominator comes free from a constant-ones row appended
#     to V (zero column in v_w + bias 1): the attn-weighted reduce
#     yields [fused_unnorm; den]; den rides the delta matmul (ow padded
#     with a unit column) + transpose, and normalization is a
#     per-partition scale on the ScalarEngine copy out of PSUM.  out_b
#     is added post-norm via a host-expanded const tile.
#   - All weights arrive in 3 packed DMAs; exp() without max-shift
#     (scores are O(1e-2)); fast-approx reciprocal (den is O(5)).

import math
import os

import numpy as np

B, L, C, W2 = 8, 1024, 64, 64
K, NTOK, D = 4, 5, 32
LCHUNK = 128
NCHUNK = L // LCHUNK
HALF = 64  # l-positions per half-chunk (320 = HALF*NTOK columns <= 512 PSUM)
NCORES = 8
NJ = 4   # 16-element w-groups per window (fp8 pair-slots: s=8 per group)
NS = 8   # PSUM w-slots per (group, c)

POOL_DR = True  # fp8 DoubleRow pooling (2 elem/partition/cycle); False = plain

# packed bf16 weight blob column offsets: ident|pw|qw|kv|ow|obx
# kv block is 96 wide: [v_w | ones-col | pad...] rows 0:33, k_w at 64:96
# (DVE operand partition windows must be 32-aligned, so v+ones sits at 0
# and k at 64)
_ID0, _PW0, _QW0, _KV0, _OW0, _OB0 = 0, 128, 160, 192, 288, 353
_WBF_COLS = 417

LAST_RESULTS = None  # BassKernelResults of the most recent run (for test.py)


def _build():
    from contextlib import ExitStack

    import concourse.bacc as bacc
    import concourse.mybir as mybir
    import concourse.tile as tile

    f32 = mybir.dt.float32
    bf16 = mybir.dt.bfloat16
    f8 = mybir.dt.float8e4
    AX = mybir.AxisListType.X
    EXP = mybir.ActivationFunctionType.Exp
    CPY = mybir.ActivationFunctionType.Copy
    DR = mybir.MatmulPerfMode.DoubleRow

    nc = bacc.Bacc(
        "TRN2",
        target_bir_lowering=False,
        debug=False,
        num_devices=NCORES,
    )

    def din(name, shape, dt=f32):
        return nc.dram_tensor(name, shape, dt, kind="ExternalInput").ap()

    tgt = din("tgt", [L, W2 * C], bf16)  # w-major [L, (w, c)]
    nbr = din("nbr", [L, K * NJ * C * 16], f8)  # [L, K, j4, C, 16w] packed
    wf8 = din("wf8", [128, 2 * 128], f8)   # pair-identity x dequant scale
    wbf = din("wbf", [128, _WBF_COLS], bf16)
    wf32 = din("wf32", [128, 3])           # pb|qb|kvb columns
    y = nc.dram_tensor("y", [L, W2 * C], bf16, kind="ExternalOutput").ap()

    with (
        tile.TileContext(nc) as tc,
        ExitStack() as ctx,
        nc.allow_low_precision(reason="fp8/bf16 staging; tolerance is 2e-2"),
    ):
        const = ctx.enter_context(tc.tile_pool(name="const", bufs=1))
        bigp = ctx.enter_context(tc.tile_pool(name="big", bufs=1))
        tokp = ctx.enter_context(tc.tile_pool(name="tok", bufs=1))
        smallp = ctx.enter_context(tc.tile_pool(name="small", bufs=2))
        ps_pool = ctx.enter_context(tc.tile_pool(name="ps_pool", bufs=1, space="PSUM"))
        ps_tt = ctx.enter_context(tc.tile_pool(name="ps_tt", bufs=1, space="PSUM"))
        ps_sm = ctx.enter_context(tc.tile_pool(name="ps_sm", bufs=2, space="PSUM"))

        identw_s = const.tile([128, 2, 128], f8)
        nc.sync.dma_start(out=identw_s[:], in_=wf8.rearrange("p (t c) -> p t c", t=2))
        wbf_s = const.tile([128, _WBF_COLS], bf16)
        nc.sync.dma_start(out=wbf_s[:], in_=wbf)
        wf32_s = const.tile([128, 3], f32)
        nc.sync.dma_start(out=wf32_s[:], in_=wf32)

        ident_s = wbf_s[:, _ID0:_PW0]
        pw_s = wbf_s[0:C, _PW0:_QW0]
        qw_s = wbf_s[0:D, _QW0:_KV0]
        kv_s = wbf_s[0:D, _KV0:_OW0]          # [D, 96]
        ow_s = wbf_s[0 : D + 1, _OW0:_OB0]    # [D+1, C+1]
        obx_s = wbf_s[:, _OB0:_WBF_COLS]      # [128, C]
        pb_s = wf32_s[0:D, 0:1]
        qb_s = wf32_s[64:96, 1:2]
        kvb_s = wf32_s[0:96, 2:3]

        # ones vectors for the score / exp-broadcast matmuls; the score
        # side lives at base partition 64 to match k's rows in the merged
        # kv output (DVE ops need equal base partitions on both inputs)
        ones96 = const.tile([96, 1], bf16)
        nc.vector.memset(ones96[64:96], 1.0)
        ones_1 = const.tile([1, D + 1], bf16)
        nc.vector.memset(ones_1[:], 1.0)

        targs = [None] * NCHUNK
        tokss = [None] * NCHUNK

        def emit_pool(i):
            l0 = i * LCHUNK
            targ = bigp.tile([LCHUNK, W2, C], bf16, tag="targ", bufs=3)
            targs[i] = targ
            nc.sync.dma_start(
                out=targ[:],
                in_=tgt[l0 : l0 + LCHUNK].rearrange("l (w c) -> l w c", c=C),
            )
            nbig = bigp.tile([LCHUNK, K, NJ, C, 16], f8, tag="nbig", bufs=3)
            nc.gpsimd.dma_start(
                out=nbig[:],
                in_=nbr[l0 : l0 + LCHUNK].rearrange(
                    "l (k j c w) -> l k j c w", k=K, j=NJ, w=16
                ),
            )

            toks = tokp.tile([LCHUNK, NTOK, C], bf16, tag="toks", bufs=2)
            tokss[i] = toks
            for half in range(2):
                pnb = ps_pool.tile(
                    [LCHUNK, 2, C * NS], f32, tag=f"pn{half}", bufs=1
                )
                for kk in range(2):
                    k = 2 * half + kk
                    for j in range(NJ):
                        if POOL_DR:
                            nc.tensor.matmul(
                                pnb[:, kk],
                                lhsT=identw_s[:],
                                rhs=nbig[:, k, j].rearrange(
                                    "l c (s two) -> l two c s", two=2
                                ),
                                start=(j == 0),
                                stop=(j == NJ - 1),
                                perf_mode=DR,
                            )
                        else:
                            for g in range(2):
                                nc.tensor.matmul(
                                    pnb[:, kk],
                                    lhsT=identw_s[:, 0],
                                    rhs=nbig[:, k, j].rearrange(
                                        "l c (s two) -> l two c s", two=2
                                    )[:, g],
                                    start=(j == 0 and g == 0),
                                    stop=(j == NJ - 1 and g == 1),
                                )
                nc.vector.reduce_sum(
                    toks[:, 1 + 2 * half : 3 + 2 * half],
                    pnb.rearrange("l k (c s) -> l k c s", s=NS),
                    axis=AX,
                )
            # target pooling from the w-major tile: slots (s8, c)
            ptg = ps_pool.tile([LCHUNK, NS * C], f32, tag="pt", bufs=1)
            for j in range(NS):
                nc.tensor.matmul(
                    ptg[:],
                    lhsT=ident_s,
                    rhs=targ[:, 8 * j : 8 * (j + 1)],
                    start=(j == 0),
                    stop=(j == NS - 1),
                )
            nc.vector.reduce_sum(
                toks[:, 0],
                ptg.rearrange("l (s c) -> l c s", c=C),
                axis=AX,
            )

        def emit_attn(i):
            l0 = i * LCHUNK
            targ = targs[i]
            toks = tokss[i]

            # ---- transpose tokens to [c, (n, l)] ----
            ps5 = ps_tt.tile([C, NTOK, LCHUNK], bf16, tag="ttp")
            for n in range(NTOK):
                nc.tensor.transpose(ps5[:, n], toks[:, n], ident_s)
            tokT = tokp.tile([C, NTOK, LCHUNK], bf16, tag="tokT", bufs=2)
            nc.scalar.copy(tokT[:], ps5[:])

            fusedT = smallp.tile([D + 1, LCHUNK], bf16)
            exps = smallp.tile([1, 2, NTOK * HALF], bf16, tag="exps")
            projs2 = []

            for h in range(2):
                lh = slice(h * HALF, (h + 1) * HALF)

                # proj = tokens @ pw + pb   -> [D, (n, l64)]
                pproj = ps_sm.tile([D, NTOK * HALF], f32, tag="sm")
                nc.tensor.matmul(pproj[:], lhsT=pw_s, rhs=tokT[:, :, lh])
                projs = smallp.tile([D, NTOK * HALF], bf16, tag="projs")
                nc.scalar.add(projs[:], pproj[:], pb_s)

                # k and v in one matmul (v widened with a ones row = den
                # accumulator at rows 0:33, k at rows 64:96);
                # q over token 0 only (contiguous slice)
                pkv = ps_sm.tile([96, NTOK * HALF], f32, tag="sm")
                nc.tensor.matmul(pkv[:], lhsT=kv_s, rhs=projs[:])
                kvs = smallp.tile([96, NTOK * HALF], bf16, tag="kvs")
                nc.scalar.add(kvs[:], pkv[:], kvb_s)

                pq = ps_sm.tile([96, HALF], f32, tag="sm")
                nc.tensor.matmul(pq[64:96], lhsT=qw_s, rhs=projs[:, 0:HALF])
                qs = smallp.tile([96, HALF], bf16, tag="qs")
                nc.scalar.add(qs[64:96], pq[64:96], qb_s)

                # scores[n, l] = sum_d q[d, l] * k[d, (n, l)]
                qk = smallp.tile([96, NTOK, HALF], bf16, tag="qk")
                nc.vector.tensor_mul(
                    qk[64:96],
                    kvs[64:96].rearrange("d (n l) -> d n l", n=NTOK),
                    qs[64:96].unsqueeze(1).to_broadcast([D, NTOK, HALF]),
                )
                psc = ps_sm.tile([1, NTOK * HALF], f32, tag="sm")
                nc.tensor.matmul(psc[:], lhsT=ones96[64:96], rhs=qk[64:96])
                # scores are O(1e-2): exp without max-shift is exact enough
                nc.scalar.activation(exps[:, h], psc[:], EXP)
                projs2.append(kvs)

            for h in range(2):
                # broadcast exp-weights over d+1 rows, weight [v; 1],
                # reduce over n -> [fused_unnorm; den]
                pab = ps_sm.tile([D + 1, NTOK * HALF], f32, tag="sm")
                nc.tensor.matmul(pab[:], lhsT=ones_1[:], rhs=exps[:, h])
                av = smallp.tile([D + 1, NTOK * HALF], bf16, tag="av")
                nc.vector.tensor_mul(av[:], projs2[h][0 : D + 1], pab[:])
                nc.vector.reduce_sum(
                    fusedT[:, h * HALF : (h + 1) * HALF],
                    av.rearrange("d (n l) -> d l n", n=NTOK),
                    axis=AX,
                )

            # delta_u = fused_u @ ow (col C carries den), then transpose;
            # normalize by 1/den per partition on the ScalarEngine, then
            # add out_b via a host-expanded const tile
            pdelta = ps_sm.tile([C + 1, LCHUNK], f32, tag="sm")
            nc.tensor.matmul(pdelta[:], lhsT=ow_s, rhs=fusedT[:])
            deltaT = smallp.tile([C + 1, LCHUNK], bf16, tag="deltaT")
            nc.scalar.copy(deltaT[:], pdelta[:])
            pdT = ps_sm.tile([LCHUNK, C + 1], bf16, tag="sm")
            nc.tensor.transpose(pdT[:], deltaT[:], ident_s[: C + 1, : C + 1])

            den_f = smallp.tile([LCHUNK, 1], f32, tag="den")
            nc.vector.tensor_copy(den_f[:], pdT[:, C : C + 1])
            rden = smallp.tile([LCHUNK, 1], f32, tag="rden")
            nc.vector.reciprocal_approx_fast(out=rden[:], in_=den_f[:])
            pdTs = smallp.tile([LCHUNK, C], bf16, tag="pdTs")
            nc.scalar.activation(pdTs[:], pdT[:, 0:C], CPY, scale=rden[:])
            nc.vector.tensor_add(pdTs[:], pdTs[:], obx_s)

            # out = target + delta (broadcast over the MIDDLE w dim -> 2x
            # DVE); halves pipeline the add against the store DMA on the
            # scalar-engine HWDGE queue (separate from the load queue)
            yv = y[l0 : l0 + LCHUNK].rearrange("l (w c) -> l w c", c=C)
            for wh in range(2):
                ws = slice(wh * (W2 // 2), (wh + 1) * (W2 // 2))
                nc.vector.tensor_add(
                    targ[:, ws],
                    targ[:, ws],
                    pdTs.unsqueeze(1).to_broadcast([LCHUNK, W2 // 2, C]),
                )
                nc.scalar.dma_start(out=yv[:, ws], in_=targ[:, ws])

        for i in range(NCHUNK):
            emit_pool(i)
            if i >= 1:
                emit_attn(i - 1)
        emit_attn(NCHUNK - 1)

    nc.compile()
    return nc


def kernel(
    target_win,
    neighbor_wins,
    proj_w,
    proj_b,
    q_w,
    q_b,
    k_w,
    k_b,
    v_w,
    v_b,
    out_w,
    out_b,
):
    global LAST_RESULTS
    import ml_dtypes

    from concourse.bass_utils import run_bass_kernel_spmd

    f = np.float32
    bf = ml_dtypes.bfloat16
    f8 = ml_dtypes.float8_e4m3

    target_win = np.asarray(target_win, f)
    neighbor_wins = np.asarray(neighbor_wins, f)

    # fp8 staging of the neighbor windows with an exact power-of-two scale
    # (dequant is baked into the pooling identity, so it costs nothing).
    amax = float(np.abs(neighbor_wins).max())
    if amax == 0.0 or not math.isfinite(amax):
        scale = 1.0
    else:
        scale = 2.0 ** min(8, max(-9, math.ceil(math.log2(amax / 224.0))))
    nbr_q = (neighbor_wins * (1.0 / scale)).astype(f8)  # [K, B, L, C, 8, 8]
    nbr_q = nbr_q.reshape(K, B, L, C, NJ, 16)

    # target in w-major [B, L, 8, 8, C] so the device add broadcasts over
    # the middle dim
    tgt_bf = np.ascontiguousarray(
        target_win.transpose(0, 1, 3, 4, 2).astype(bf)
    )

    identw = np.zeros((128, 2, 128), f8)
    identw[np.arange(128), :, np.arange(128)] = f8(scale)

    # Fold the window-mean (1/64) into proj_w and the 1/sqrt(D) score
    # scale into q_w/q_b (linear ops commute with these scalings).
    pw = np.asarray(proj_w, f) / float(W2)
    sc = 1.0 / math.sqrt(D)
    qw = np.asarray(q_w, f) * sc
    qb = np.asarray(q_b, f) * sc
    # [v_w | ones-col | pad | k_w]: v widened with a constant-ones row
    # (zero weight column + bias 1) that accumulates the softmax
    # denominator; k sits at rows 64:96 of the matmul output so every
    # DVE read window is 32-partition aligned.
    kv_ext = np.zeros((D, 96), f)
    kv_ext[:, :D] = np.asarray(v_w, f)
    kv_ext[:, 64:96] = np.asarray(k_w, f)
    kvb_ext = np.zeros((96,), f)
    kvb_ext[:D] = np.asarray(v_b, f)
    kvb_ext[D] = 1.0
    kvb_ext[64:96] = np.asarray(k_b, f)
    # ow padded so the den row rides the delta matmul + transpose.
    ow_ext = np.zeros((D + 1, C + 1), f)
    ow_ext[:D, :C] = np.asarray(out_w, f)
    ow_ext[D, C] = 1.0

    wbf = np.zeros((128, _WBF_COLS), bf)
    wbf[:, _ID0:_PW0] = np.eye(128, dtype=bf)
    wbf[0:C, _PW0:_QW0] = pw.astype(bf)
    wbf[0:D, _QW0:_KV0] = qw.astype(bf)
    wbf[0:D, _KV0:_OW0] = kv_ext.astype(bf)
    wbf[0 : D + 1, _OW0:_OB0] = ow_ext.astype(bf)
    wbf[:, _OB0:_WBF_COLS] = np.asarray(out_b, f).astype(bf)[None, :]

    wf32 = np.zeros((128, 3), f)
    wf32[0:D, 0] = np.asarray(proj_b, f)
    wf32[64:96, 1] = qb  # q lives at base partition 64 (matches k rows)
    wf32[0:96, 2] = kvb_ext

    shared = {
        "wf8": identw.reshape(128, 256),
        "wbf": wbf,
        "wf32": wf32,
    }
    in_maps = []
    for b in range(NCORES):
        in_maps.append(
            {
                "tgt": tgt_bf[b].reshape(L, W2 * C),
                # [K, L, C, j, 16] -> [L, K, j, C, 16]
                "nbr": np.ascontiguousarray(
                    nbr_q[:, b].transpose(1, 0, 3, 2, 4)
                ).reshape(L, K * NJ * C * 16),
                **shared,
            }
        )

    nc = _build()
    res = run_bass_kernel_spmd(
        nc,
        in_maps,
        list(range(NCORES)),
        trace=bool(os.environ.get("KERNEL_PROFILE")),
    )
    LAST_RESULTS = res
    # y is bf16 w-major [L, (w, c)] -> [L, C, 8, 8] f32
    out = np.stack(
        [
            res.results[b]["y"]
            .astype(np.float32)
            .reshape(L, 8, 8, C)
            .transpose(0, 3, 1, 2)
            for b in range(NCORES)
        ]
    )
    return np.ascontiguousarray(out)


# revision 17
# speedup vs baseline: 1.0007x; 1.0007x over previous
# Trainium2 Bass kernel for CrossScaleFreqAttention.
#
# Math (per batch b):
#   tokens[l, n, c] = mean over the 8x8 window of {target, 4 neighbors}[l, c]
#   proj = tokens @ proj_w + proj_b ; q/k/v linear ; softmax over n (5)
#   delta[l, c] = (attn-weighted v) @ out_w + out_b
#   out = target_win + delta broadcast over the window
#
# Sharding: data-parallel over B=8 -> one batch element per NeuronCore,
# weights replicated, no cross-core communication.
#
# Memory-regime kernel. The harness tolerance is 2e-2 and the attention
# delta is ~0.1% of the output magnitude, so the big tensors are staged
# at reduced precision on the host (all compute stays on device):
#   - neighbor windows -> fp8 e4m3 with a power-of-two per-tensor scale
#     (dequant exact, baked into the pooling matmul stationary weights)
#   - target windows   -> bf16, w-major [L, W2, C] layout (so the final
#     delta broadcast-add has unit stride innermost -> 2x DVE mode)
#   - output           -> bf16 w-major store, fixed up on the host
# Per-core HBM traffic: 100.7 MB (f32) -> 33.6 MB  (~94 us roofline at
# ~360 GB/s/core).  Measured rel err of the whole scheme: ~3.9e-3.
#
# Structure notes:
#   - The chunk loop is software-pipelined: pooling for chunk i is
#     emitted BEFORE attention for chunk i-1, so the TensorEngine's
#     in-order stream always has dense pool matmuls to chew on while the
#     serial attention chain of the previous chunk resolves on
#     Scalar/Vector.  (Without this the PE idles >3.4 us per chunk and
#     the HAM clock gate re-throttles it to 1.2 GHz - measured 2x.)
#   - ALL pooling on the TensorEngine as 512-column matmuls. Neighbors:
#     fp8 DoubleRow (2 elem/partition/cycle), PSUM slots (c, s8), two
#     half-tiles (k01/k23) so the VectorE fold of one half overlaps the
#     matmuls of the other.  Target windows pool in plain bf16 from the
#     w-major tile (slots (s8, c)).
#   - Attention columns are (n, l)-ordered: the token-transpose PSUM
#     [c, n, l] copies out flat, q is a contiguous slice, and the qk
#     product broadcasts q over the MIDDLE dim (2x DVE).  k and v share
#     one matmul + one bias-add ([k_w | v_w] packed, 65 output rows).
#   - Softmax denominator comes free from a constant-ones row appended
#     to V (zero column in v_w + bias 1): the attn-weighted reduce
#     yields [fused_unnorm; den]; den rides the delta matmul (ow padded
#     with a unit column) + transpose, and normalization is a
#     per-partition scale on the ScalarEngine copy out of PSUM.  out_b
#     is added post-norm via a host-expanded const tile.
#   - All weights arrive in 3 packed DMAs; exp() without max-shift
#     (scores are O(1e-2)); fast-approx reciprocal (den is O(5)).

import math
import os

import numpy as np

B, L, C, W2 = 8, 1024, 64, 64
K, NTOK, D = 4, 5, 32
LCHUNK = 128
NCHUNK = L // LCHUNK
HALF = 64  # l-positions per half-chunk (320 = HALF*NTOK columns <= 512 PSUM)
NCORES = 8
NJ = 4   # 16-element w-groups per window (fp8 pair-slots: s=8 per group)
NS = 8   # PSUM w-slots per (group, c)

POOL_DR = True  # fp8 DoubleRow pooling (2 elem/partition/cycle); False = plain

# packed bf16 weight blob column offsets: ident|pw|qw|kv|ow|obx
# kv block is 96 wide: [v_w | ones-col | pad...] rows 0:33, k_w at 64:96
# (DVE operand partition windows must be 32-aligned, so v+ones sits at 0
# and k at 64)
_ID0, _PW0, _QW0, _KV0, _OW0, _OB0 = 0, 128, 160, 192, 288, 353
_WBF_COLS = 417

LAST_RESULTS = None  # BassKernelResults of the most recent run (for test.py)


def _build():
    from contextlib import ExitStack

    import concourse.bacc as bacc
    import concourse.mybir as mybir
    import concourse.tile as tile

    f32 = mybir.dt.float32
    bf16 = mybir.dt.bfloat16
    f8 = mybir.dt.float8e4
    AX = mybir.AxisListType.X
    EXP = mybir.ActivationFunctionType.Exp
    CPY = mybir.ActivationFunctionType.Copy
    DR = mybir.MatmulPerfMode.DoubleRow

    nc = bacc.Bacc(
        "TRN2",
        target_bir_lowering=False,
        debug=False,
        num_devices=NCORES,
    )

    def din(name, shape, dt=f32):
        return nc.dram_tensor(name, shape, dt, kind="ExternalInput").ap()

    tgt = din("tgt", [L, W2 * C], bf16)  # w-major [L, (w, c)]
    nbr = din("nbr", [L, K * NJ * C * 16], f8)  # [L, K, j4, C, 16w] packed
    wf8 = din("wf8", [128, 2 * 128], f8)   # pair-identity x dequant scale
    wbf = din("wbf", [128, _WBF_COLS], bf16)
    wf32 = din("wf32", [128, 3])           # pb|qb|kvb columns
    y = nc.dram_tensor("y", [L, W2 * C], bf16, kind="ExternalOutput").ap()

    with (
        tile.TileContext(nc) as tc,
        ExitStack() as ctx,
        nc.allow_low_precision(reason="fp8/bf16 staging; tolerance is 2e-2"),
    ):
        const = ctx.enter_context(tc.tile_pool(name="const", bufs=1))
        bigp = ctx.enter_context(tc.tile_pool(name="big", bufs=1))
        tokp = ctx.enter_context(tc.tile_pool(name="tok", bufs=1))
        smallp = ctx.enter_context(tc.tile_pool(name="small", bufs=2))
        ps_pool = ctx.enter_context(tc.tile_pool(name="ps_pool", bufs=1, space="PSUM"))
        ps_tt = ctx.enter_context(tc.tile_pool(name="ps_tt", bufs=1, space="PSUM"))
        ps_sm = ctx.enter_context(tc.tile_pool(name="ps_sm", bufs=2, space="PSUM"))

        identw_s = const.tile([128, 2, 128], f8)
        nc.sync.dma_start(out=identw_s[:], in_=wf8.rearrange("p (t c) -> p t c", t=2))
        wbf_s = const.tile([128, _WBF_COLS], bf16)
        nc.sync.dma_start(out=wbf_s[:], in_=wbf)
        wf32_s = const.tile([128, 3], f32)
        nc.sync.dma_start(out=wf32_s[:], in_=wf32)

        ident_s = wbf_s[:, _ID0:_PW0]
        pw_s = wbf_s[0:C, _PW0:_QW0]
        qw_s = wbf_s[0:D, _QW0:_KV0]
        kv_s = wbf_s[0:D, _KV0:_OW0]          # [D, 96]
        ow_s = wbf_s[0 : D + 1, _OW0:_OB0]    # [D+1, C+1]
        obx_s = wbf_s[:, _OB0:_WBF_COLS]      # [128, C]
        pb_s = wf32_s[0:D, 0:1]
        qb_s = wf32_s[64:96, 1:2]
        kvb_s = wf32_s[0:96, 2:3]

        # ones vectors for the score / exp-broadcast matmuls; the score
        # side lives at base partition 64 to match k's rows in the merged
        # kv output (DVE ops need equal base partitions on both inputs)
        ones96 = const.tile([96, 1], bf16)
        nc.vector.memset(ones96[64:96], 1.0)
        ones_1 = const.tile([1, D + 1], bf16)
        nc.vector.memset(ones_1[:], 1.0)

        targs = [None] * NCHUNK
        tokss = [None] * NCHUNK

        def emit_pool(i):
            l0 = i * LCHUNK
            targ = bigp.tile([LCHUNK, W2, C], bf16, tag="targ", bufs=2)
            targs[i] = targ
            nc.sync.dma_start(
                out=targ[:],
                in_=tgt[l0 : l0 + LCHUNK].rearrange("l (w c) -> l w c", c=C),
            )
            nbig = bigp.tile([LCHUNK, K, NJ, C, 16], f8, tag="nbig", bufs=2)
            nc.gpsimd.dma_start(
                out=nbig[:],
                in_=nbr[l0 : l0 + LCHUNK].rearrange(
                    "l (k j c w) -> l k j c w", k=K, j=NJ, w=16
                ),
            )

            toks = tokp.tile([LCHUNK, NTOK, C], bf16, tag="toks", bufs=2)
            tokss[i] = toks
            for half in range(2):
                pnb = ps_pool.tile(
                    [LCHUNK, 2, C * NS], f32, tag=f"pn{half}", bufs=1
                )
                for kk in range(2):
                    k = 2 * half + kk
                    for j in range(NJ):
                        if POOL_DR:
                            nc.tensor.matmul(
                                pnb[:, kk],
                                lhsT=identw_s[:],
                                rhs=nbig[:, k, j].rearrange(
                                    "l c (s two) -> l two c s", two=2
                                ),
                                start=(j == 0),
                                stop=(j == NJ - 1),
                                perf_mode=DR,
                            )
                        else:
                            for g in range(2):
                                nc.tensor.matmul(
                                    pnb[:, kk],
                                    lhsT=identw_s[:, 0],
                                    rhs=nbig[:, k, j].rearrange(
                                        "l c (s two) -> l two c s", two=2
                                    )[:, g],
                                    start=(j == 0 and g == 0),
                                    stop=(j == NJ - 1 and g == 1),
                                )
                nc.vector.reduce_sum(
                    toks[:, 1 + 2 * half : 3 + 2 * half],
                    pnb.rearrange("l k (c s) -> l k c s", s=NS),
                    axis=AX,
                )
            # target pooling from the w-major tile: slots (s8, c)
            ptg = ps_pool.tile([LCHUNK, NS * C], f32, tag="pt", bufs=1)
            for j in range(NS):
                nc.tensor.matmul(
                    ptg[:],
                    lhsT=ident_s,
                    rhs=targ[:, 8 * j : 8 * (j + 1)],
                    start=(j == 0),
                    stop=(j == NS - 1),
                )
            nc.vector.reduce_sum(
                toks[:, 0],
                ptg.rearrange("l (s c) -> l c s", c=C),
                axis=AX,
            )

        def emit_attn(i):
            l0 = i * LCHUNK
            targ = targs[i]
            toks = tokss[i]

            # ---- transpose tokens to [c, (n, l)] ----
            ps5 = ps_tt.tile([C, NTOK, LCHUNK], bf16, tag="ttp")
            for n in range(NTOK):
                nc.tensor.transpose(ps5[:, n], toks[:, n], ident_s)
            tokT = tokp.tile([C, NTOK, LCHUNK], bf16, tag="tokT", bufs=2)
            nc.scalar.copy(tokT[:], ps5[:])

            fusedT = smallp.tile([D + 1, LCHUNK], bf16)
            exps = smallp.tile([1, 2, NTOK * HALF], bf16, tag="exps")
            projs2 = []

            for h in range(2):
                lh = slice(h * HALF, (h + 1) * HALF)

                # proj = tokens @ pw + pb   -> [D, (n, l64)]
                pproj = ps_sm.tile([D, NTOK * HALF], f32, tag="sm")
                nc.tensor.matmul(pproj[:], lhsT=pw_s, rhs=tokT[:, :, lh])
                projs = smallp.tile([D, NTOK * HALF], bf16, tag="projs")
                nc.scalar.add(projs[:], pproj[:], pb_s)

                # k and v in one matmul (v widened with a ones row = den
                # accumulator at rows 0:33, k at rows 64:96);
                # q over token 0 only (contiguous slice)
                pkv = ps_sm.tile([96, NTOK * HALF], f32, tag="sm")
                nc.tensor.matmul(pkv[:], lhsT=kv_s, rhs=projs[:])
                kvs = smallp.tile([96, NTOK * HALF], bf16, tag="kvs")
                nc.scalar.add(kvs[:], pkv[:], kvb_s)

                pq = ps_sm.tile([96, HALF], f32, tag="sm")
                nc.tensor.matmul(pq[64:96], lhsT=qw_s, rhs=projs[:, 0:HALF])
                qs = smallp.tile([96, HALF], bf16, tag="qs")
                nc.scalar.add(qs[64:96], pq[64:96], qb_s)

                # scores[n, l] = sum_d q[d, l] * k[d, (n, l)]
                qk = smallp.tile([96, NTOK, HALF], bf16, tag="qk")
                nc.vector.tensor_mul(
                    qk[64:96],
                    kvs[64:96].rearrange("d (n l) -> d n l", n=NTOK),
                    qs[64:96].unsqueeze(1).to_broadcast([D, NTOK, HALF]),
                )
                psc = ps_sm.tile([1, NTOK * HALF], f32, tag="sm")
                nc.tensor.matmul(psc[:], lhsT=ones96[64:96], rhs=qk[64:96])
                # scores are O(1e-2): exp without max-shift is exact enough
                nc.scalar.activation(exps[:, h], psc[:], EXP)
                projs2.append(kvs)

            for h in range(2):
                # broadcast exp-weights over d+1 rows, weight [v; 1],
                # reduce over n -> [fused_unnorm; den]
                pab = ps_sm.tile([D + 1, NTOK * HALF], f32, tag="sm")
                nc.tensor.matmul(pab[:], lhsT=ones_1[:], rhs=exps[:, h])
                av = smallp.tile([D + 1, NTOK * HALF], bf16, tag="av")
                nc.vector.tensor_mul(av[:], projs2[h][0 : D + 1], pab[:])
                nc.vector.reduce_sum(
                    fusedT[:, h * HALF : (h + 1) * HALF],
                    av.rearrange("d (n l) -> d l n", n=NTOK),
                    axis=AX,
                )

            # delta_u = fused_u @ ow (col C carries den), then transpose;
            # normalize by 1/den per partition on the ScalarEngine, then
            # add out_b via a host-expanded const tile
            pdelta = ps_sm.tile([C + 1, LCHUNK], f32, tag="sm")
            nc.tensor.matmul(pdelta[:], lhsT=ow_s, rhs=fusedT[:])
            deltaT = smallp.tile([C + 1, LCHUNK], bf16, tag="deltaT")
            nc.scalar.copy(deltaT[:], pdelta[:])
            pdT = ps_sm.tile([LCHUNK, C + 1], bf16, tag="sm")
            nc.tensor.transpose(pdT[:], deltaT[:], ident_s[: C + 1, : C + 1])

            den_f = smallp.tile([LCHUNK, 1], f32, tag="den")
            nc.vector.tensor_copy(den_f[:], pdT[:, C : C + 1])
            rden = smallp.tile([LCHUNK, 1], f32, tag="rden")
            nc.vector.reciprocal_approx_fast(out=rden[:], in_=den_f[:])
            pdTs = smallp.tile([LCHUNK, C], bf16, tag="pdTs")
            nc.scalar.activation(pdTs[:], pdT[:, 0:C], CPY, scale=rden[:])
            nc.vector.tensor_add(pdTs[:], pdTs[:], obx_s)

            # out = target + delta (broadcast over the MIDDLE w dim -> 2x
            # DVE); halves pipeline the add against the store DMA on the
            # scalar-engine HWDGE queue (separate from the load queue)
            yv = y[l0 : l0 + LCHUNK].rearrange("l (w c) -> l w c", c=C)
            for wh in range(2):
                ws = slice(wh * (W2 // 2), (wh + 1) * (W2 // 2))
                nc.vector.tensor_add(
                    targ[:, ws],
                    targ[:, ws],
                    pdTs.unsqueeze(1).to_broadcast([LCHUNK, W2 // 2, C]),
                )
                nc.scalar.dma_start(out=yv[:, ws], in_=targ[:, ws])

        for i in range(NCHUNK):
            emit_pool(i)
            if i >= 1:
                emit_attn(i - 1)
        emit_attn(NCHUNK - 1)

    nc.compile()
    return nc


def kernel(
    target_win,
    neighbor_wins,
    proj_w,
    proj_b,
    q_w,
    q_b,
    k_w,
    k_b,
    v_w,
    v_b,
    out_w,
    out_b,
):
    global LAST_RESULTS
    import ml_dtypes

    from concourse.bass_utils import run_bass_kernel_spmd

    f = np.float32
    bf = ml_dtypes.bfloat16
    f8 = ml_dtypes.float8_e4m3

    target_win = np.asarray(target_win, f)
    neighbor_wins = np.asarray(neighbor_wins, f)

    # fp8 staging of the neighbor windows with an exact power-of-two scale
    # (dequant is baked into the pooling identity, so it costs nothing).
    amax = float(np.abs(neighbor_wins).max())
    if amax == 0.0 or not math.isfinite(amax):
        scale = 1.0
    else:
        scale = 2.0 ** min(8, max(-9, math.ceil(math.log2(amax / 224.0))))
    nbr_q = (neighbor_wins * (1.0 / scale)).astype(f8)  # [K, B, L, C, 8, 8]
    nbr_q = nbr_q.reshape(K, B, L, C, NJ, 16)

    # target in w-major [B, L, 8, 8, C] so the device add broadcasts over
    # the middle dim
    tgt_bf = np.ascontiguousarray(
        target_win.transpose(0, 1, 3, 4, 2).astype(bf)
    )

    identw = np.zeros((128, 2, 128), f8)
    identw[np.arange(128), :, np.arange(128)] = f8(scale)

    # Fold the window-mean (1/64) into proj_w and the 1/sqrt(D) score
    # scale into q_w/q_b (linear ops commute with these scalings).
    pw = np.asarray(proj_w, f) / float(W2)
    sc = 1.0 / math.sqrt(D)
    qw = np.asarray(q_w, f) * sc
    qb = np.asarray(q_b, f) * sc
    # [v_w | ones-col | pad | k_w]: v widened with a constant-ones row
    # (zero weight column + bias 1) that accumulates the softmax
    # denominator; k sits at rows 64:96 of the matmul output so every
    # DVE read window is 32-partition aligned.
    kv_ext = np.zeros((D, 96), f)
    kv_ext[:, :D] = np.asarray(v_w, f)
    kv_ext[:, 64:96] = np.asarray(k_w, f)
    kvb_ext = np.zeros((96,), f)
    kvb_ext[:D] = np.asarray(v_b, f)
    kvb_ext[D] = 1.0
    kvb_ext[64:96] = np.asarray(k_b, f)
    # ow padded so the den row rides the delta matmul + transpose.
    ow_ext = np.zeros((D + 1, C + 1), f)
    ow_ext[:D, :C] = np.asarray(out_w, f)
    ow_ext[D, C] = 1.0

    wbf = np.zeros((128, _WBF_COLS), bf)
    wbf[:, _ID0:_PW0] = np.eye(128, dtype=bf)
    wbf[0:C, _PW0:_QW0] = pw.astype(bf)
    wbf[0:D, _QW0:_KV0] = qw.astype(bf)
    wbf[0:D, _KV0:_OW0] = kv_ext.astype(bf)
    wbf[0 : D + 1, _OW0:_OB0] = ow_ext.astype(bf)
    wbf[:, _OB0:_WBF_COLS] = np.asarray(out_b, f).astype(bf)[None, :]

    wf32 = np.zeros((128, 3), f)
    wf32[0:D, 0] = np.asarray(proj_b, f)
    wf32[64:96, 1] = qb  # q lives at base partition 64 (matches k rows)
    wf32[0:96, 2] = kvb_ext

    shared = {
        "wf8": identw.reshape(128, 256),
        "wbf": wbf,
        "wf32": wf32,
    }
    in_maps = []
    for b in range(NCORES):
        in_maps.append(
            {
                "tgt": tgt_bf[b].reshape(L, W2 * C),
                # [K, L, C, j, 16] -> [L, K, j, C, 16]
                "nbr": np.ascontiguousarray(
                    nbr_q[:, b].transpose(1, 0, 3, 2, 4)
                ).reshape(L, K * NJ * C * 16),
                **shared,
            }
        )

    nc = _build()
    res = run_bass_kernel_spmd(
        nc,
        in_maps,
        list(range(NCORES)),
        trace=bool(os.environ.get("KERNEL_PROFILE")),
    )
    LAST_RESULTS = res
    # y is bf16 w-major [L, (w, c)] -> [L, C, 8, 8] f32
    out = np.stack(
        [
            res.results[b]["y"]
            .astype(np.float32)
            .reshape(L, 8, 8, C)
            .transpose(0, 3, 1, 2)
            for b in range(NCORES)
        ]
    )
    return np.ascontiguousarray(out)
